# revision 13
# baseline (speedup 1.0000x reference)
"""Causal self-attention (B=2, T=2048, C=1024, H=16) on 8 Trainium2 cores.

Sharding: DP2 over batch x TP4 over heads (4 heads/core). Each core computes
its batch's QKV projection for its heads, RoPE, causal attention, and a
partial c_proj over its 256 input channels. Host sums the 4 partials per
batch and adds b_proj.

All matmuls run in float32r (full PE rate, ~1.5e-4 rounding). q/k weight rows
are deinterleaved on the host so RoPE's even/odd pair operations become
contiguous 32-row blocks; the RoPE "swap" is a +-1 permutation matmul on the
PE. qkv projection emits qT/kT/vT as [dims, t]; vT is PE-transposed into
v[t, dims] with a ones column appended per head so the attention row-sum
(softmax denominator) falls out of the same matmul as p@v (M=65). Scores are
computed two heads at a time via tile_position row packing, exp runs on
ScalarE straight from PSUM with the 1/sqrt(D) scale fused, and the causal
mask is an affine_select on the diagonal blocks only (off-diagonal dead
columns are never computed).
"""

import sys

sys.path.insert(0, "/opt/trn_rl_repo")

import math

import numpy as np

import concourse.bass as bass
import concourse.mybir as mybir
import concourse.tile as tile
from concourse import bacc, bass_utils

B, T, C = 2, 2048, 1024
H, D = 16, 64
N_CORES = 8
DP, TP = 2, 4
HPC = H // TP  # heads per core
SC = 512  # t-chunk width / psum bank width
NT = T // SC
NSB = T // 128  # s-blocks

F32 = mybir.dt.float32
F32R = mybir.dt.float32r

_cached = {}


def _build_program():
    nc = bacc.Bacc("TRN2", target_bir_lowering=False, debug=False, num_devices=N_CORES)

    xT_d = nc.dram_tensor("xT", [C, T], F32, kind="ExternalInput").ap()
    wqk_d = nc.dram_tensor("wqk", [C, 512], F32, kind="ExternalInput").ap()
    wv_d = nc.dram_tensor("wv", [C, 256], F32, kind="ExternalInput").ap()
    wpT_d = nc.dram_tensor("wpT", [256, C], F32, kind="ExternalInput").ap()
    bqk_d = nc.dram_tensor("bqk", [4, 128], F32, kind="ExternalInput").ap()
    bv_d = nc.dram_tensor("bv", [1, 256], F32, kind="ExternalInput").ap()
    cos_d = nc.dram_tensor("cosT", [128, T], F32, kind="ExternalInput").ap()
    sin_d = nc.dram_tensor("sinT", [128, T], F32, kind="ExternalInput").ap()
    psw_d = nc.dram_tensor("pswapT", [128, 128], F32, kind="ExternalInput").ap()
    out_d = nc.dram_tensor("out", [T, C], F32, kind="ExternalOutput").ap()

    with tile.TileContext(nc) as tc:
        with (
            tc.tile_pool(name="const", bufs=1) as const,
            tc.tile_pool(name="rotp", bufs=1) as rotp,
            tc.tile_pool(name="vsbp", bufs=1) as vsbp,
        ):
            psw_sb = const.tile([128, 128], F32R)
            cos_sb = const.tile([128, T], F32)
            sin_sb = const.tile([128, T], F32)
            bqk_sb = const.tile([128, 4], F32)
            bv_row = const.tile([1, 256], F32)
            bv_bc = const.tile([128, 256], F32)
            wpT_sb = const.tile([128, 2, C], F32R)

            def load_consts():
                # issued after the first x/w tiles so phase A starts sooner
                nc.sync.dma_start(out=psw_sb[:], in_=psw_d[:, :].bitcast(F32R))
                nc.sync.dma_start(out=cos_sb[:], in_=cos_d[:, :])
                nc.sync.dma_start(out=sin_sb[:], in_=sin_d[:, :])
                nc.sync.dma_start(out=bqk_sb[:], in_=bqk_d.rearrange("a b -> b a"))
                nc.sync.dma_start(out=bv_row[:], in_=bv_d[:, :])
                nc.gpsimd.partition_broadcast(bv_bc[:, :], bv_row[0:1, :])
                nc.sync.dma_start(
                    out=wpT_sb[:],
                    in_=wpT_d.rearrange("(a b) c -> b a c", b=128).bitcast(F32R),
                )

            # qT/kT after rope: m=0,1 q head-pairs; m=2,3 k head-pairs
            rot = [
                rotp.tile([128, T], F32R, tag=f"rot{m}", name=f"rot{m}")
                for m in range(4)
            ]
            # v with ones column per head: [128part(t), NSB, HPC*65]
            v_sb = vsbp.tile([128, NSB, HPC * 65], F32R)
            nc.vector.memset(v_sb[:].bitcast(F32), 1.0)

            # ---------------- Phase A: QKV projection + RoPE ----------------
            with (
                tc.tile_pool(name="wqkp", bufs=1) as wqkp,
                tc.tile_pool(name="wvp", bufs=1) as wvp,
                tc.tile_pool(name="xchp", bufs=2) as xchp,
                tc.tile_pool(name="rawp", bufs=1) as rawp,
                tc.tile_pool(name="ttmp", bufs=3) as ttmp,
                tc.tile_pool(name="psA", bufs=3, space="PSUM") as psA,
                tc.tile_pool(name="psV", bufs=2, space="PSUM") as psV,
                tc.tile_pool(name="psW", bufs=2, space="PSUM") as psW,
            ):
                wqk_sb = wqkp.tile([128, 8, 512], F32R)
                wv_sb = wvp.tile([128, 8, 256], F32R)
                wqk_r = wqk_d.rearrange("(a b) c -> b a c", b=128).bitcast(F32R)
                wv_r = wv_d.rearrange("(a b) c -> b a c", b=128).bitcast(F32R)
                raw = [
                    rawp.tile([128, T], F32R, tag=f"raw{m}", name=f"raw{m}")
                    for m in range(4)
                ]
                xT_r = xT_d.rearrange("(a b) c -> b a c", b=128).bitcast(F32R)

                # split loads across issuing engines so descriptor issue
                # (~1us per dma_start on one engine) doesn't serialize, and
                # halve so the first 4-ct chain can start early
                xch0 = xchp.tile([128, 8, SC], F32R, tag="xch", name="xch0")
                nc.sync.dma_start(out=wqk_sb[:, 0:4, :], in_=wqk_r[:, 0:4, :])
                nc.gpsimd.dma_start(out=xch0[:, 0:4, :], in_=xT_r[:, 0:4, 0:SC])
                nc.sync.dma_start(out=wqk_sb[:, 4:8, :], in_=wqk_r[:, 4:8, :])
                nc.gpsimd.dma_start(out=xch0[:, 4:8, :], in_=xT_r[:, 4:8, 0:SC])
                nc.sync.dma_start(out=wv_sb[:], in_=wv_r[:, :, :])
                load_consts()

                for nch in range(4):
                    sl = bass.ts(nch, SC)
                    if nch == 0:
                        xch = xch0
                    else:
                        xch = xchp.tile([128, 8, SC], F32R, tag="xch")
                        nc.sync.dma_start(out=xch[:], in_=xT_r[:, :, sl])
                    # q,k projection: out[m-tile, t-chunk]
                    for m in range(4):
                        ps = psA.tile([128, SC], F32, tag="psqk")
                        for ct in range(8):
                            nc.tensor.matmul(
                                ps[:],
                                wqk_sb[:, ct, bass.ts(m, 128)],
                                xch[:, ct, :],
                                start=(ct == 0),
                                stop=(ct == 7),
                            )
                        nc.scalar.activation(
                            out=raw[m][:, sl],
                            in_=ps[:],
                            func=mybir.ActivationFunctionType.Identity,
                            bias=bqk_sb[:, m : m + 1],
                        )
                    # v projection for the 4 t-subtiles of this chunk
                    for tml in range(4):
                        tm = nch * 4 + tml
                        psv = psV.tile([128, 256], F32, tag="psv")
                        for ct in range(8):
                            nc.tensor.matmul(
                                psv[:],
                                xch[:, ct, bass.ts(tml, 128)],
                                wv_sb[:, ct, :],
                                start=(ct == 0),
                                stop=(ct == 7),
                            )
                        nc.vector.tensor_add(
                            v_sb[:, tm, :]
                            .rearrange("p (h c) -> p h c", h=HPC)[:, :, 0:64],
                            psv[:].rearrange("p (h c) -> p h c", h=HPC),
                            bv_bc[:].rearrange("p (h c) -> p h c", h=HPC),
                        )
                    # rope on the 4 qk tiles for this chunk
                    for m in range(4):
                        psw = psW.tile([128, SC], F32, tag="psw")
                        nc.tensor.matmul(psw[:], psw_sb[:], raw[m][:, sl])
                        tmp = ttmp.tile([128, SC], F32, tag="ttmp")
                        nc.vector.tensor_mul(tmp[:], psw[:], sin_sb[:, sl])
                        nc.vector.tensor_mul(
                            rot[m][:, sl], raw[m][:, sl].bitcast(F32), cos_sb[:, sl]
                        )
                        nc.gpsimd.tensor_add(
                            rot[m][:, sl], rot[m][:, sl].bitcast(F32), tmp[:]
                        )

            # ---------------- Phase B: attention + c_proj per t-chunk ----------------
            with (
                tc.tile_pool(name="ptp", bufs=4) as ptp,
                tc.tile_pool(name="ypairp", bufs=4) as ypairp,
                tc.tile_pool(name="ysbp", bufs=4) as ysbp,
                tc.tile_pool(name="lrowp", bufs=4) as lrowp,
                tc.tile_pool(name="bcp", bufs=4) as bcp,
                tc.tile_pool(name="ostp", bufs=3) as ostp,
                tc.tile_pool(name="psS", bufs=2, space="PSUM") as psS,
                tc.tile_pool(name="psY", bufs=1, space="PSUM") as psY,
                tc.tile_pool(name="psO", bufs=1, space="PSUM") as psO,
            ):
                for tci in range(NT):
                    t0 = tci * SC
                    nsb = tci * 4 + 4
                    ypair = [
                        ypairp.tile([128, SC], F32R, tag=f"yp{p}", name=f"yp{p}")
                        for p in range(2)
                    ]
                    for p in range(2):
                        psy = [
                            psY.tile([65, SC], F32, tag=f"psy{q}", name=f"psy{q}")
                            for q in range(2)
                        ]
                        for sbi in range(nsb):
                            s0 = sbi * 128
                            ssl = bass.ds(s0, 128)
                            # cols below d0 are causally dead: never computed
                            d0 = max(0, s0 - t0)
                            nn = SC - d0
                            # both heads' scores in one 2-bank psum tile
                            pss = psS.tile([128, 2 * SC], F32, tag="pss")
                            nc.tensor.matmul(
                                pss[:, d0:SC],
                                rot[2 + p][0:64, ssl],
                                rot[p][0:64, bass.ds(t0 + d0, nn)],
                                tile_position=(0, 0),
                            )
                            nc.tensor.matmul(
                                pss[:, SC + d0 : 2 * SC],
                                rot[2 + p][64:128, ssl],
                                rot[p][64:128, bass.ds(t0 + d0, nn)],
                                tile_position=(64, 0),
                            )
                            pt = ptp.tile([128, 2 * SC], F32R, tag="pt")
                            pt3 = pt[:].rearrange("p (h c) -> p h c", h=2)[:, :, d0:SC]
                            nc.scalar.activation(
                                out=pt3,
                                in_=pss[:].rearrange("p (h c) -> p h c", h=2)[
                                    :, :, d0:SC
                                ],
                                func=mybir.ActivationFunctionType.Exp,
                                scale=1.0 / math.sqrt(D),
                            )
                            if s0 >= t0:
                                # zero t < s for both heads: keep y' - x >= 0
                                nc.gpsimd.affine_select(
                                    out=pt3,
                                    in_=pt3,
                                    compare_op=mybir.AluOpType.is_ge,
                                    fill=0.0,
                                    base=0,
                                    pattern=[[0, 2], [1, nn]],
                                    channel_multiplier=-1,
                                )
                            for q in range(2):
                                h = 2 * p + q
                                nc.tensor.matmul(
                                    psy[q][:, d0:SC],
                                    v_sb[:, sbi, h * 65 : h * 65 + 65],
                                    pt[:, q * SC + d0 : (q + 1) * SC],
                                    start=(sbi == 0),
                                    stop=(sbi == nsb - 1),
                                )
                        for q in range(2):
                            # free the psum bank right away; l-pipeline runs from SBUF
                            ysb = ysbp.tile([65, SC], F32, tag="ysb")
                            if q == 0:
                                nc.scalar.copy(ysb[:, :], psy[q][:, :])
                            else:
                                nc.vector.tensor_copy(ysb[:, :], psy[q][:, :])
                            lraw = lrowp.tile([1, SC], F32, tag="lraw")
                            nc.vector.tensor_copy(lraw[0:1, :], ysb[64:65, :])
                            lrow0 = lrowp.tile([1, SC], F32, tag="lrow0")
                            nc.vector.reciprocal_approx_fast(lrow0[0:1, :], lraw[0:1, :])
                            bc = bcp.tile([64, SC], F32, tag="bc")
                            nc.gpsimd.partition_broadcast(bc[:, :], lrow0[0:1, :])
                            nc.vector.tensor_mul(
                                ypair[p][q * 64 : (q + 1) * 64, :],
                                ysb[0:64, :],
                                bc[:, :],
                            )
                    # c_proj partial for this chunk
                    for ms in range(4):
                        pso = psO.tile([128, C], F32, tag="pso")
                        for kp in range(2):
                            for nch2 in range(2):
                                nc.tensor.matmul(
                                    pso[:, bass.ts(nch2, 512)],
                                    ypair[kp][:, bass.ts(ms, 128)],
                                    wpT_sb[:, kp, bass.ts(nch2, 512)],
                                    start=(kp == 0),
                                    stop=(kp == 1),
                                )
                        ost = ostp.tile([128, C], F32, tag="ost")
                        if ms % 2 == 0:
                            nc.scalar.copy(ost[:], pso[:])
                        else:
                            nc.vector.tensor_copy(ost[:], pso[:])
                        nc.sync.dma_start(
                            out=out_d[bass.ds(t0 + ms * 128, 128), :], in_=ost[:]
                        )

    nc.compile()
    return nc


def _host_shards(x, w_attn, b_attn, w_proj):
    """Per-core input dicts. Core c: batch c//TP, heads [HPC*(c%TP) .. )."""
    pos = np.arange(T, dtype=np.float64)
    div = np.exp(np.arange(0, D, 2, dtype=np.float64) * (-(math.log(10000.0) / D)))
    sinu = np.outer(pos, div)  # [T, 32]
    cosT = np.tile(np.cos(sinu).T, (4, 1)).astype(np.float32)  # [128, T]
    sinT = np.tile(np.sin(sinu).T, (4, 1)).astype(np.float32)

    psw = np.zeros((128, 128), dtype=np.float32)  # P[out,in]
    for blk in (0, 64):
        for j in range(32):
            psw[blk + j, blk + 32 + j] = -1.0
            psw[blk + 32 + j, blk + j] = 1.0
    pswapT = np.ascontiguousarray(psw.T)

    ev = np.arange(0, D, 2)
    od = np.arange(1, D, 2)
    in_maps = []
    for c in range(N_CORES):
        b, lane = c // TP, c % TP
        heads = [HPC * lane + i for i in range(HPC)]
        idx_qk = []
        for off in (0, C):  # q rows then k rows, deinterleaved per head
            for p in range(2):
                for hh in (heads[2 * p], heads[2 * p + 1]):
                    base = off + hh * D
                    idx_qk.extend((base + ev).tolist())
                    idx_qk.extend((base + od).tolist())
        idx_qk = np.array(idx_qk)
        idx_v = np.concatenate([2 * C + h * D + np.arange(D) for h in heads])
        cols_p = np.concatenate([h * D + np.arange(D) for h in heads])
        in_maps.append(
            {
                "xT": np.ascontiguousarray(x[b].T),
                "wqk": np.ascontiguousarray(w_attn[idx_qk, :].T),
                "wv": np.ascontiguousarray(w_attn[idx_v, :].T),
                "wpT": np.ascontiguousarray(w_proj[:, cols_p].T),
                "bqk": np.ascontiguousarray(b_attn[idx_qk].reshape(4, 128)),
                "bv": np.ascontiguousarray(b_attn[idx_v].reshape(1, 256)),
                "cosT": cosT,
                "sinT": sinT,
                "pswapT": pswapT,
            }
        )
    return in_maps


def kernel(x, w_attn, b_attn, w_proj, b_proj, _trace=False):
    x = np.asarray(x, dtype=np.float32)
    w_attn = np.asarray(w_attn, dtype=np.float32)
    b_attn = np.asarray(b_attn, dtype=np.float32)
    w_proj = np.asarray(w_proj, dtype=np.float32)
    b_proj = np.asarray(b_proj, dtype=np.float32)

    if "nc" not in _cached:
        _cached["nc"] = _build_program()
    nc = _cached["nc"]

    in_maps = _host_shards(x, w_attn, b_attn, w_proj)
    res = bass_utils.run_bass_kernel_spmd(
        nc, in_maps, core_ids=list(range(N_CORES)), trace=_trace
    )
    _cached["last_result"] = res

    out = np.empty((B, T, C), dtype=np.float32)
    for b in range(B):
        acc = res.results[b * TP]["out"].astype(np.float32).copy()
        for lane in range(1, TP):
            acc += res.results[b * TP + lane]["out"]
        out[b] = acc + b_proj[None, :]
    return out


# revision 14
# speedup vs baseline: 1.0185x; 1.0185x over previous
"""Causal self-attention (B=2, T=2048, C=1024, H=16) on 8 Trainium2 cores.

Sharding: DP2 over batch x TP4 over heads (4 heads/core). Each core computes
its batch's QKV projection for its heads, RoPE, causal attention, and a
partial c_proj over its 256 input channels. Host sums the 4 partials per
batch and adds b_proj.

All matmuls run in float32r (full PE rate, ~1.5e-4 rounding). q/k weight rows
are deinterleaved on the host so RoPE's even/odd pair operations become
contiguous 32-row blocks; the RoPE "swap" is a +-1 permutation matmul on the
PE. qkv projection emits qT/kT/vT as [dims, t]; vT is PE-transposed into
v[t, dims] with a ones column appended per head so the attention row-sum
(softmax denominator) falls out of the same matmul as p@v (M=65). Scores are
computed two heads at a time via tile_position row packing, exp runs on
ScalarE straight from PSUM with the 1/sqrt(D) scale fused, and the causal
mask is an affine_select on the diagonal blocks only (off-diagonal dead
columns are never computed).
"""

import sys

sys.path.insert(0, "/opt/trn_rl_repo")

import math

import numpy as np

import concourse.bass as bass
import concourse.mybir as mybir
import concourse.tile as tile
from concourse import bacc, bass_utils

B, T, C = 2, 2048, 1024
H, D = 16, 64
N_CORES = 8
DP, TP = 2, 4
HPC = H // TP  # heads per core
SC = 512  # t-chunk width / psum bank width
NT = T // SC
NSB = T // 128  # s-blocks

F32 = mybir.dt.float32
F32R = mybir.dt.float32r

_cached = {}


def _build_program():
    nc = bacc.Bacc("TRN2", target_bir_lowering=False, debug=False, num_devices=N_CORES)

    xT_d = nc.dram_tensor("xT", [C, T], F32, kind="ExternalInput").ap()
    wqk_d = nc.dram_tensor("wqk", [C, 512], F32, kind="ExternalInput").ap()
    wv_d = nc.dram_tensor("wv", [C, 256], F32, kind="ExternalInput").ap()
    wpT_d = nc.dram_tensor("wpT", [256, C], F32, kind="ExternalInput").ap()
    bqk_d = nc.dram_tensor("bqk", [4, 128], F32, kind="ExternalInput").ap()
    bv_d = nc.dram_tensor("bv", [1, 256], F32, kind="ExternalInput").ap()
    cos_d = nc.dram_tensor("cosT", [128, T], F32, kind="ExternalInput").ap()
    sin_d = nc.dram_tensor("sinT", [128, T], F32, kind="ExternalInput").ap()
    psw_d = nc.dram_tensor("pswapT", [128, 128], F32, kind="ExternalInput").ap()
    out_d = nc.dram_tensor("out", [T, C], F32, kind="ExternalOutput").ap()

    with tile.TileContext(nc) as tc:
        with (
            tc.tile_pool(name="const", bufs=1) as const,
            tc.tile_pool(name="rotp", bufs=1) as rotp,
            tc.tile_pool(name="vsbp", bufs=1) as vsbp,
        ):
            psw_sb = const.tile([128, 128], F32R)
            cos_sb = const.tile([128, T], F32)
            sin_sb = const.tile([128, T], F32)
            bqk_sb = const.tile([128, 4], F32)
            bv_row = const.tile([1, 256], F32)
            bv_bc = const.tile([128, 256], F32)
            wpT_sb = const.tile([128, 2, C], F32R)

            def load_consts():
                # issued after the first x/w tiles so phase A starts sooner
                nc.sync.dma_start(out=psw_sb[:], in_=psw_d[:, :].bitcast(F32R))
                nc.sync.dma_start(out=cos_sb[:], in_=cos_d[:, :])
                nc.sync.dma_start(out=sin_sb[:], in_=sin_d[:, :])
                nc.sync.dma_start(out=bqk_sb[:], in_=bqk_d.rearrange("a b -> b a"))
                nc.sync.dma_start(out=bv_row[:], in_=bv_d[:, :])
                nc.gpsimd.partition_broadcast(bv_bc[:, :], bv_row[0:1, :])
                nc.sync.dma_start(
                    out=wpT_sb[:],
                    in_=wpT_d.rearrange("(a b) c -> b a c", b=128).bitcast(F32R),
                )

            # qT/kT after rope: m=0,1 q head-pairs; m=2,3 k head-pairs
            rot = [
                rotp.tile([128, T], F32R, tag=f"rot{m}", name=f"rot{m}")
                for m in range(4)
            ]
            # v with ones column per head: [128part(t), NSB, HPC*65]
            v_sb = vsbp.tile([128, NSB, HPC * 65], F32R)
            nc.vector.memset(v_sb[:].bitcast(F32), 1.0)

            # ---------------- Phase A: QKV projection + RoPE ----------------
            with (
                tc.tile_pool(name="wqkp", bufs=1) as wqkp,
                tc.tile_pool(name="wvp", bufs=1) as wvp,
                tc.tile_pool(name="xchp", bufs=2) as xchp,
                tc.tile_pool(name="rawp", bufs=1) as rawp,
                tc.tile_pool(name="ttmp", bufs=3) as ttmp,
                tc.tile_pool(name="psA", bufs=3, space="PSUM") as psA,
                tc.tile_pool(name="psV", bufs=2, space="PSUM") as psV,
                tc.tile_pool(name="psW", bufs=2, space="PSUM") as psW,
            ):
                wqk_sb = wqkp.tile([128, 8, 512], F32R)
                wv_sb = wvp.tile([128, 8, 256], F32R)
                wqk_r = wqk_d.rearrange("(a b) c -> b a c", b=128).bitcast(F32R)
                wv_r = wv_d.rearrange("(a b) c -> b a c", b=128).bitcast(F32R)
                raw = [
                    rawp.tile([128, T], F32R, tag=f"raw{m}", name=f"raw{m}")
                    for m in range(4)
                ]
                xT_r = xT_d.rearrange("(a b) c -> b a c", b=128).bitcast(F32R)

                # split loads across issuing engines so descriptor issue
                # (~1us per dma_start on one engine) doesn't serialize, and
                # halve so the first 4-ct chain can start early
                xch0 = xchp.tile([128, 8, SC], F32R, tag="xch", name="xch0")
                for ct in range(8):
                    nc.sync.dma_start(out=wqk_sb[:, ct, :], in_=wqk_r[:, ct, :])
                    nc.gpsimd.dma_start(out=xch0[:, ct, :], in_=xT_r[:, ct, 0:SC])
                nc.sync.dma_start(out=wv_sb[:], in_=wv_r[:, :, :])
                load_consts()

                for nch in range(4):
                    sl = bass.ts(nch, SC)
                    if nch == 0:
                        xch = xch0
                    else:
                        xch = xchp.tile([128, 8, SC], F32R, tag="xch")
                        nc.sync.dma_start(out=xch[:], in_=xT_r[:, :, sl])
                    # q,k projection: out[m-tile, t-chunk]
                    for m in range(4):
                        ps = psA.tile([128, SC], F32, tag="psqk")
                        for ct in range(8):
                            nc.tensor.matmul(
                                ps[:],
                                wqk_sb[:, ct, bass.ts(m, 128)],
                                xch[:, ct, :],
                                start=(ct == 0),
                                stop=(ct == 7),
                            )
                        nc.scalar.activation(
                            out=raw[m][:, sl],
                            in_=ps[:],
                            func=mybir.ActivationFunctionType.Identity,
                            bias=bqk_sb[:, m : m + 1],
                        )
                    # v projection for the 4 t-subtiles of this chunk
                    for tml in range(4):
                        tm = nch * 4 + tml
                        psv = psV.tile([128, 256], F32, tag="psv")
                        for ct in range(8):
                            nc.tensor.matmul(
                                psv[:],
                                xch[:, ct, bass.ts(tml, 128)],
                                wv_sb[:, ct, :],
                                start=(ct == 0),
                                stop=(ct == 7),
                            )
                        nc.vector.tensor_add(
                            v_sb[:, tm, :]
                            .rearrange("p (h c) -> p h c", h=HPC)[:, :, 0:64],
                            psv[:].rearrange("p (h c) -> p h c", h=HPC),
                            bv_bc[:].rearrange("p (h c) -> p h c", h=HPC),
                        )
                    # rope on the 4 qk tiles for this chunk
                    for m in range(4):
                        psw = psW.tile([128, SC], F32, tag="psw")
                        nc.tensor.matmul(psw[:], psw_sb[:], raw[m][:, sl])
                        tmp = ttmp.tile([128, SC], F32, tag="ttmp")
                        nc.vector.tensor_mul(tmp[:], psw[:], sin_sb[:, sl])
                        nc.vector.tensor_mul(
                            rot[m][:, sl], raw[m][:, sl].bitcast(F32), cos_sb[:, sl]
                        )
                        nc.vector.tensor_add(
                            rot[m][:, sl], rot[m][:, sl].bitcast(F32), tmp[:]
                        )

            # ---------------- Phase B: attention + c_proj per t-chunk ----------------
            with (
                tc.tile_pool(name="ptp", bufs=4) as ptp,
                tc.tile_pool(name="ypairp", bufs=4) as ypairp,
                tc.tile_pool(name="ysbp", bufs=4) as ysbp,
                tc.tile_pool(name="lrowp", bufs=4) as lrowp,
                tc.tile_pool(name="bcp", bufs=4) as bcp,
                tc.tile_pool(name="ostp", bufs=3) as ostp,
                tc.tile_pool(name="psS", bufs=2, space="PSUM") as psS,
                tc.tile_pool(name="psY", bufs=1, space="PSUM") as psY,
                tc.tile_pool(name="psO", bufs=1, space="PSUM") as psO,
            ):
                for tci in range(NT):
                    t0 = tci * SC
                    nsb = tci * 4 + 4
                    ypair = [
                        ypairp.tile([128, SC], F32R, tag=f"yp{p}", name=f"yp{p}")
                        for p in range(2)
                    ]
                    for p in range(2):
                        psy = [
                            psY.tile([65, SC], F32, tag=f"psy{q}", name=f"psy{q}")
                            for q in range(2)
                        ]
                        for sbi in range(nsb):
                            s0 = sbi * 128
                            ssl = bass.ds(s0, 128)
                            # cols below d0 are causally dead: never computed
                            d0 = max(0, s0 - t0)
                            nn = SC - d0
                            # both heads' scores in one 2-bank psum tile
                            pss = psS.tile([128, 2 * SC], F32, tag="pss")
                            nc.tensor.matmul(
                                pss[:, d0:SC],
                                rot[2 + p][0:64, ssl],
                                rot[p][0:64, bass.ds(t0 + d0, nn)],
                                tile_position=(0, 0),
                            )
                            nc.tensor.matmul(
                                pss[:, SC + d0 : 2 * SC],
                                rot[2 + p][64:128, ssl],
                                rot[p][64:128, bass.ds(t0 + d0, nn)],
                                tile_position=(64, 0),
                            )
                            pt = ptp.tile([128, 2 * SC], F32R, tag="pt")
                            pt3 = pt[:].rearrange("p (h c) -> p h c", h=2)[:, :, d0:SC]
                            nc.scalar.activation(
                                out=pt3,
                                in_=pss[:].rearrange("p (h c) -> p h c", h=2)[
                                    :, :, d0:SC
                                ],
                                func=mybir.ActivationFunctionType.Exp,
                                scale=1.0 / math.sqrt(D),
                            )
                            if s0 >= t0:
                                # zero t < s for both heads: keep y' - x >= 0
                                nc.gpsimd.affine_select(
                                    out=pt3,
                                    in_=pt3,
                                    compare_op=mybir.AluOpType.is_ge,
                                    fill=0.0,
                                    base=0,
                                    pattern=[[0, 2], [1, nn]],
                                    channel_multiplier=-1,
                                )
                            for q in range(2):
                                h = 2 * p + q
                                nc.tensor.matmul(
                                    psy[q][:, d0:SC],
                                    v_sb[:, sbi, h * 65 : h * 65 + 65],
                                    pt[:, q * SC + d0 : (q + 1) * SC],
                                    start=(sbi == 0),
                                    stop=(sbi == nsb - 1),
                                )
                        for q in range(2):
                            # free the psum bank right away; l-pipeline runs from SBUF
                            ysb = ysbp.tile([65, SC], F32, tag="ysb")
                            if q == 0:
                                nc.scalar.copy(ysb[:, :], psy[q][:, :])
                            else:
                                nc.vector.tensor_copy(ysb[:, :], psy[q][:, :])
                            lraw = lrowp.tile([1, SC], F32, tag="lraw")
                            nc.vector.tensor_copy(lraw[0:1, :], ysb[64:65, :])
                            lrow0 = lrowp.tile([1, SC], F32, tag="lrow0")
                            nc.vector.reciprocal_approx_fast(lrow0[0:1, :], lraw[0:1, :])
                            bc = bcp.tile([64, SC], F32, tag="bc")
                            nc.gpsimd.partition_broadcast(bc[:, :], lrow0[0:1, :])
                            nc.vector.tensor_mul(
                                ypair[p][q * 64 : (q + 1) * 64, :],
                                ysb[0:64, :],
                                bc[:, :],
                            )
                    # c_proj partial for this chunk
                    for ms in range(4):
                        pso = psO.tile([128, C], F32, tag="pso")
                        for kp in range(2):
                            for nch2 in range(2):
                                nc.tensor.matmul(
                                    pso[:, bass.ts(nch2, 512)],
                                    ypair[kp][:, bass.ts(ms, 128)],
                                    wpT_sb[:, kp, bass.ts(nch2, 512)],
                                    start=(kp == 0),
                                    stop=(kp == 1),
                                )
                        ost = ostp.tile([128, C], F32, tag="ost")
                        # split the psum drain across both engines to free pso fast
                        nc.scalar.copy(ost[:, 0:512], pso[:, 0:512])
                        nc.vector.tensor_copy(ost[:, 512:1024], pso[:, 512:1024])
                        nc.sync.dma_start(
                            out=out_d[bass.ds(t0 + ms * 128, 128), :], in_=ost[:]
                        )

    nc.compile()
    return nc


def _host_shards(x, w_attn, b_attn, w_proj):
    """Per-core input dicts. Core c: batch c//TP, heads [HPC*(c%TP) .. )."""
    pos = np.arange(T, dtype=np.float64)
    div = np.exp(np.arange(0, D, 2, dtype=np.float64) * (-(math.log(10000.0) / D)))
    sinu = np.outer(pos, div)  # [T, 32]
    cosT = np.tile(np.cos(sinu).T, (4, 1)).astype(np.float32)  # [128, T]
    sinT = np.tile(np.sin(sinu).T, (4, 1)).astype(np.float32)

    psw = np.zeros((128, 128), dtype=np.float32)  # P[out,in]
    for blk in (0, 64):
        for j in range(32):
            psw[blk + j, blk + 32 + j] = -1.0
            psw[blk + 32 + j, blk + j] = 1.0
    pswapT = np.ascontiguousarray(psw.T)

    ev = np.arange(0, D, 2)
    od = np.arange(1, D, 2)
    in_maps = []
    for c in range(N_CORES):
        b, lane = c // TP, c % TP
        heads = [HPC * lane + i for i in range(HPC)]
        idx_qk = []
        for off in (0, C):  # q rows then k rows, deinterleaved per head
            for p in range(2):
                for hh in (heads[2 * p], heads[2 * p + 1]):
                    base = off + hh * D
                    idx_qk.extend((base + ev).tolist())
                    idx_qk.extend((base + od).tolist())
        idx_qk = np.array(idx_qk)
        idx_v = np.concatenate([2 * C + h * D + np.arange(D) for h in heads])
        cols_p = np.concatenate([h * D + np.arange(D) for h in heads])
        in_maps.append(
            {
                "xT": np.ascontiguousarray(x[b].T),
                "wqk": np.ascontiguousarray(w_attn[idx_qk, :].T),
                "wv": np.ascontiguousarray(w_attn[idx_v, :].T),
                "wpT": np.ascontiguousarray(w_proj[:, cols_p].T),
                "bqk": np.ascontiguousarray(b_attn[idx_qk].reshape(4, 128)),
                "bv": np.ascontiguousarray(b_attn[idx_v].reshape(1, 256)),
                "cosT": cosT,
                "sinT": sinT,
                "pswapT": pswapT,
            }
        )
    return in_maps


def kernel(x, w_attn, b_attn, w_proj, b_proj, _trace=False):
    x = np.asarray(x, dtype=np.float32)
    w_attn = np.asarray(w_attn, dtype=np.float32)
    b_attn = np.asarray(b_attn, dtype=np.float32)
    w_proj = np.asarray(w_proj, dtype=np.float32)
    b_proj = np.asarray(b_proj, dtype=np.float32)

    if "nc" not in _cached:
        _cached["nc"] = _build_program()
    nc = _cached["nc"]

    in_maps = _host_shards(x, w_attn, b_attn, w_proj)
    res = bass_utils.run_bass_kernel_spmd(
        nc, in_maps, core_ids=list(range(N_CORES)), trace=_trace
    )
    _cached["last_result"] = res

    out = np.empty((B, T, C), dtype=np.float32)
    for b in range(B):
        acc = res.results[b * TP]["out"].astype(np.float32).copy()
        for lane in range(1, TP):
            acc += res.results[b * TP + lane]["out"]
        out[b] = acc + b_proj[None, :]
    return out


# revision 15
# speedup vs baseline: 1.0422x; 1.0233x over previous
"""Causal self-attention (B=2, T=2048, C=1024, H=16) on 8 Trainium2 cores.

Sharding: DP2 over batch x TP4 over heads (4 heads/core). Each core computes
its batch's QKV projection for its heads, RoPE, causal attention, and a
partial c_proj over its 256 input channels. Host sums the 4 partials per
batch and adds b_proj.

All matmuls run in float32r (full PE rate, ~1.5e-4 rounding). q/k weight rows
are deinterleaved on the host so RoPE's even/odd pair operations become
contiguous 32-row blocks; the RoPE "swap" is a +-1 permutation matmul on the
PE. qkv projection emits qT/kT/vT as [dims, t]; vT is PE-transposed into
v[t, dims] with a ones column appended per head so the attention row-sum
(softmax denominator) falls out of the same matmul as p@v (M=65). Scores are
computed two heads at a time via tile_position row packing, exp runs on
ScalarE straight from PSUM with the 1/sqrt(D) scale fused, and the causal
mask is an affine_select on the diagonal blocks only (off-diagonal dead
columns are never computed).
"""

import sys

sys.path.insert(0, "/opt/trn_rl_repo")

import math

import numpy as np

import concourse.bass as bass
import concourse.mybir as mybir
import concourse.tile as tile
from concourse import bacc, bass_utils

B, T, C = 2, 2048, 1024
H, D = 16, 64
N_CORES = 8
DP, TP = 2, 4
HPC = H // TP  # heads per core
SC = 512  # t-chunk width / psum bank width
NT = T // SC
NSB = T // 128  # s-blocks

F32 = mybir.dt.float32
F32R = mybir.dt.float32r

_cached = {}


def _build_program():
    nc = bacc.Bacc("TRN2", target_bir_lowering=False, debug=False, num_devices=N_CORES)

    xT_d = nc.dram_tensor("xT", [C, T], F32, kind="ExternalInput").ap()
    wqk_d = nc.dram_tensor("wqk", [C, 512], F32, kind="ExternalInput").ap()
    wv_d = nc.dram_tensor("wv", [C, 256], F32, kind="ExternalInput").ap()
    wpT_d = nc.dram_tensor("wpT", [256, C], F32, kind="ExternalInput").ap()
    bqk_d = nc.dram_tensor("bqk", [4, 128], F32, kind="ExternalInput").ap()
    bv_d = nc.dram_tensor("bv", [1, 256], F32, kind="ExternalInput").ap()
    cos_d = nc.dram_tensor("cosT", [128, T], F32, kind="ExternalInput").ap()
    sin_d = nc.dram_tensor("sinT", [128, T], F32, kind="ExternalInput").ap()
    psw_d = nc.dram_tensor("pswapT", [128, 128], F32, kind="ExternalInput").ap()
    out_d = nc.dram_tensor("out", [T, C], F32, kind="ExternalOutput").ap()

    with tile.TileContext(nc) as tc:
        with (
            tc.tile_pool(name="const", bufs=1) as const,
            tc.tile_pool(name="rotp", bufs=1) as rotp,
            tc.tile_pool(name="vsbp", bufs=1) as vsbp,
        ):
            psw_sb = const.tile([128, 128], F32R)
            cos_sb = const.tile([128, T], F32)
            sin_sb = const.tile([128, T], F32)
            bqk_sb = const.tile([128, 4], F32)
            bv_row = const.tile([1, 256], F32)
            bv_bc = const.tile([128, 256], F32)
            wpT_sb = const.tile([128, 2, C], F32R)

            def load_consts():
                # issued after the first x/w tiles so phase A starts sooner
                nc.sync.dma_start(out=psw_sb[:], in_=psw_d[:, :].bitcast(F32R))
                nc.sync.dma_start(out=cos_sb[:], in_=cos_d[:, :])
                nc.sync.dma_start(out=sin_sb[:], in_=sin_d[:, :])
                nc.sync.dma_start(out=bqk_sb[:], in_=bqk_d.rearrange("a b -> b a"))
                nc.sync.dma_start(out=bv_row[:], in_=bv_d[:, :])
                nc.gpsimd.partition_broadcast(bv_bc[:, :], bv_row[0:1, :])
                nc.sync.dma_start(
                    out=wpT_sb[:],
                    in_=wpT_d.rearrange("(a b) c -> b a c", b=128).bitcast(F32R),
                )

            # qT/kT after rope: m=0,1 q head-pairs; m=2,3 k head-pairs
            rot = [
                rotp.tile([128, T], F32R, tag=f"rot{m}", name=f"rot{m}")
                for m in range(4)
            ]
            # v with ones column per head: [128part(t), NSB, HPC*65]
            v_sb = vsbp.tile([128, NSB, HPC * 65], F32R)
            nc.vector.memset(v_sb[:].bitcast(F32), 1.0)

            # ---------------- Phase A: QKV projection + RoPE ----------------
            with (
                tc.tile_pool(name="wqkp", bufs=1) as wqkp,
                tc.tile_pool(name="wvp", bufs=1) as wvp,
                tc.tile_pool(name="xchp", bufs=2) as xchp,
                tc.tile_pool(name="rawp", bufs=1) as rawp,
                tc.tile_pool(name="ttmp", bufs=3) as ttmp,
                tc.tile_pool(name="psA", bufs=3, space="PSUM") as psA,
                tc.tile_pool(name="psV", bufs=2, space="PSUM") as psV,
                tc.tile_pool(name="psW", bufs=2, space="PSUM") as psW,
            ):
                wqk_sb = wqkp.tile([128, 8, 512], F32R)
                wv_sb = wvp.tile([128, 8, 256], F32R)
                wqk_r = wqk_d.rearrange("(a b) c -> b a c", b=128).bitcast(F32R)
                wv_r = wv_d.rearrange("(a b) c -> b a c", b=128).bitcast(F32R)
                raw = [
                    rawp.tile([128, T], F32R, tag=f"raw{m}", name=f"raw{m}")
                    for m in range(4)
                ]
                xT_r = xT_d.rearrange("(a b) c -> b a c", b=128).bitcast(F32R)

                # split loads across issuing engines so descriptor issue
                # (~1us per dma_start on one engine) doesn't serialize, and
                # halve so the first 4-ct chain can start early
                xch0 = xchp.tile([128, 8, SC], F32R, tag="xch", name="xch0")
                for ct in range(8):
                    nc.sync.dma_start(out=wqk_sb[:, ct, :], in_=wqk_r[:, ct, :])
                    nc.gpsimd.dma_start(out=xch0[:, ct, :], in_=xT_r[:, ct, 0:SC])
                nc.sync.dma_start(out=wv_sb[:], in_=wv_r[:, :, :])
                load_consts()

                for nch in range(4):
                    sl = bass.ts(nch, SC)
                    if nch == 0:
                        xch = xch0
                    else:
                        xch = xchp.tile([128, 8, SC], F32R, tag="xch")
                        nc.sync.dma_start(out=xch[:], in_=xT_r[:, :, sl])
                    # q,k projection: out[m-tile, t-chunk]
                    for m in range(4):
                        ps = psA.tile([128, SC], F32, tag="psqk")
                        for ct in range(8):
                            nc.tensor.matmul(
                                ps[:],
                                wqk_sb[:, ct, bass.ts(m, 128)],
                                xch[:, ct, :],
                                start=(ct == 0),
                                stop=(ct == 7),
                            )
                        nc.scalar.activation(
                            out=raw[m][:, sl],
                            in_=ps[:],
                            func=mybir.ActivationFunctionType.Identity,
                            bias=bqk_sb[:, m : m + 1],
                        )
                    # v projection for the 4 t-subtiles of this chunk
                    for tml in range(4):
                        tm = nch * 4 + tml
                        psv = psV.tile([128, 256], F32, tag="psv")
                        for ct in range(8):
                            nc.tensor.matmul(
                                psv[:],
                                xch[:, ct, bass.ts(tml, 128)],
                                wv_sb[:, ct, :],
                                start=(ct == 0),
                                stop=(ct == 7),
                            )
                        nc.vector.tensor_add(
                            v_sb[:, tm, :]
                            .rearrange("p (h c) -> p h c", h=HPC)[:, :, 0:64],
                            psv[:].rearrange("p (h c) -> p h c", h=HPC),
                            bv_bc[:].rearrange("p (h c) -> p h c", h=HPC),
                        )
                    # rope on the 4 qk tiles for this chunk
                    for m in range(4):
                        psw = psW.tile([128, SC], F32, tag="psw")
                        nc.tensor.matmul(psw[:], psw_sb[:], raw[m][:, sl])
                        tmp = ttmp.tile([128, SC], F32, tag="ttmp")
                        nc.vector.tensor_mul(tmp[:], psw[:], sin_sb[:, sl])
                        nc.vector.tensor_mul(
                            rot[m][:, sl], raw[m][:, sl].bitcast(F32), cos_sb[:, sl]
                        )
                        nc.vector.tensor_add(
                            rot[m][:, sl], rot[m][:, sl].bitcast(F32), tmp[:]
                        )

            # ---------------- Phase B: attention + c_proj per t-chunk ----------------
            with (
                tc.tile_pool(name="ptp", bufs=6) as ptp,
                tc.tile_pool(name="ypairp", bufs=4) as ypairp,
                tc.tile_pool(name="ysbp", bufs=4) as ysbp,
                tc.tile_pool(name="lrowp", bufs=4) as lrowp,
                tc.tile_pool(name="bcp", bufs=4) as bcp,
                tc.tile_pool(name="ostp", bufs=3) as ostp,
                tc.tile_pool(name="psS", bufs=2, space="PSUM") as psS,
                tc.tile_pool(name="psY", bufs=1, space="PSUM") as psY,
                tc.tile_pool(name="psO", bufs=2, space="PSUM") as psO,
            ):
                for tci in range(NT):
                    t0 = tci * SC
                    nsb = tci * 4 + 4
                    ypair = [
                        ypairp.tile([128, SC], F32R, tag=f"yp{p}", name=f"yp{p}")
                        for p in range(2)
                    ]
                    for p in range(2):
                        psy = [
                            psY.tile([65, SC], F32, tag=f"psy{q}", name=f"psy{q}")
                            for q in range(2)
                        ]
                        for sbi in range(nsb):
                            s0 = sbi * 128
                            ssl = bass.ds(s0, 128)
                            # cols below d0 are causally dead: never computed
                            d0 = max(0, s0 - t0)
                            nn = SC - d0
                            # both heads' scores in one 2-bank psum tile
                            pss = psS.tile([128, 2 * SC], F32, tag="pss")
                            nc.tensor.matmul(
                                pss[:, d0:SC],
                                rot[2 + p][0:64, ssl],
                                rot[p][0:64, bass.ds(t0 + d0, nn)],
                                tile_position=(0, 0),
                            )
                            nc.tensor.matmul(
                                pss[:, SC + d0 : 2 * SC],
                                rot[2 + p][64:128, ssl],
                                rot[p][64:128, bass.ds(t0 + d0, nn)],
                                tile_position=(64, 0),
                            )
                            pt = ptp.tile([128, 2 * SC], F32R, tag="pt")
                            pt3 = pt[:].rearrange("p (h c) -> p h c", h=2)[:, :, d0:SC]
                            nc.scalar.activation(
                                out=pt3,
                                in_=pss[:].rearrange("p (h c) -> p h c", h=2)[
                                    :, :, d0:SC
                                ],
                                func=mybir.ActivationFunctionType.Exp,
                                scale=1.0 / math.sqrt(D),
                            )
                            if s0 >= t0:
                                # zero t < s for both heads: keep y' - x >= 0
                                nc.gpsimd.affine_select(
                                    out=pt3,
                                    in_=pt3,
                                    compare_op=mybir.AluOpType.is_ge,
                                    fill=0.0,
                                    base=0,
                                    pattern=[[0, 2], [1, nn]],
                                    channel_multiplier=-1,
                                )
                            for q in range(2):
                                h = 2 * p + q
                                nc.tensor.matmul(
                                    psy[q][:, d0:SC],
                                    v_sb[:, sbi, h * 65 : h * 65 + 65],
                                    pt[:, q * SC + d0 : (q + 1) * SC],
                                    start=(sbi == 0),
                                    stop=(sbi == nsb - 1),
                                )
                        for q in range(2):
                            # free the psum bank right away; l-pipeline runs from SBUF
                            ysb = ysbp.tile([65, SC], F32, tag="ysb")
                            if q == 0:
                                nc.scalar.copy(ysb[:, :], psy[q][:, :])
                            else:
                                nc.vector.tensor_copy(ysb[:, :], psy[q][:, :])
                            lraw = lrowp.tile([1, SC], F32, tag="lraw")
                            nc.vector.tensor_copy(lraw[0:1, :], ysb[64:65, :])
                            lrow0 = lrowp.tile([1, SC], F32, tag="lrow0")
                            nc.vector.reciprocal_approx_fast(lrow0[0:1, :], lraw[0:1, :])
                            bc = bcp.tile([64, SC], F32, tag="bc")
                            nc.gpsimd.partition_broadcast(bc[:, :], lrow0[0:1, :])
                            nc.vector.tensor_mul(
                                ypair[p][q * 64 : (q + 1) * 64, :],
                                ysb[0:64, :],
                                bc[:, :],
                            )
                    # c_proj partial for this chunk; two 1-bank slots pipeline
                    for ms in range(4):
                        ost = ostp.tile([128, C], F32, tag="ost")
                        for nch2 in range(2):
                            pso = psO.tile([128, 512], F32, tag="pso")
                            for kp in range(2):
                                nc.tensor.matmul(
                                    pso[:],
                                    ypair[kp][:, bass.ts(ms, 128)],
                                    wpT_sb[:, kp, bass.ts(nch2, 512)],
                                    start=(kp == 0),
                                    stop=(kp == 1),
                                )
                            if nch2 == 0:
                                nc.scalar.copy(ost[:, 0:512], pso[:])
                            else:
                                nc.vector.tensor_copy(ost[:, 512:1024], pso[:])
                        nc.sync.dma_start(
                            out=out_d[bass.ds(t0 + ms * 128, 128), :], in_=ost[:]
                        )

    nc.compile()
    return nc


def _host_shards(x, w_attn, b_attn, w_proj):
    """Per-core input dicts. Core c: batch c//TP, heads [HPC*(c%TP) .. )."""
    pos = np.arange(T, dtype=np.float64)
    div = np.exp(np.arange(0, D, 2, dtype=np.float64) * (-(math.log(10000.0) / D)))
    sinu = np.outer(pos, div)  # [T, 32]
    cosT = np.tile(np.cos(sinu).T, (4, 1)).astype(np.float32)  # [128, T]
    sinT = np.tile(np.sin(sinu).T, (4, 1)).astype(np.float32)

    psw = np.zeros((128, 128), dtype=np.float32)  # P[out,in]
    for blk in (0, 64):
        for j in range(32):
            psw[blk + j, blk + 32 + j] = -1.0
            psw[blk + 32 + j, blk + j] = 1.0
    pswapT = np.ascontiguousarray(psw.T)

    ev = np.arange(0, D, 2)
    od = np.arange(1, D, 2)
    in_maps = []
    for c in range(N_CORES):
        b, lane = c // TP, c % TP
        heads = [HPC * lane + i for i in range(HPC)]
        idx_qk = []
        for off in (0, C):  # q rows then k rows, deinterleaved per head
            for p in range(2):
                for hh in (heads[2 * p], heads[2 * p + 1]):
                    base = off + hh * D
                    idx_qk.extend((base + ev).tolist())
                    idx_qk.extend((base + od).tolist())
        idx_qk = np.array(idx_qk)
        idx_v = np.concatenate([2 * C + h * D + np.arange(D) for h in heads])
        cols_p = np.concatenate([h * D + np.arange(D) for h in heads])
        in_maps.append(
            {
                "xT": np.ascontiguousarray(x[b].T),
                "wqk": np.ascontiguousarray(w_attn[idx_qk, :].T),
                "wv": np.ascontiguousarray(w_attn[idx_v, :].T),
                "wpT": np.ascontiguousarray(w_proj[:, cols_p].T),
                "bqk": np.ascontiguousarray(b_attn[idx_qk].reshape(4, 128)),
                "bv": np.ascontiguousarray(b_attn[idx_v].reshape(1, 256)),
                "cosT": cosT,
                "sinT": sinT,
                "pswapT": pswapT,
            }
        )
    return in_maps


def kernel(x, w_attn, b_attn, w_proj, b_proj, _trace=False):
    x = np.asarray(x, dtype=np.float32)
    w_attn = np.asarray(w_attn, dtype=np.float32)
    b_attn = np.asarray(b_attn, dtype=np.float32)
    w_proj = np.asarray(w_proj, dtype=np.float32)
    b_proj = np.asarray(b_proj, dtype=np.float32)

    if "nc" not in _cached:
        _cached["nc"] = _build_program()
    nc = _cached["nc"]

    in_maps = _host_shards(x, w_attn, b_attn, w_proj)
    res = bass_utils.run_bass_kernel_spmd(
        nc, in_maps, core_ids=list(range(N_CORES)), trace=_trace
    )
    _cached["last_result"] = res

    out = np.empty((B, T, C), dtype=np.float32)
    for b in range(B):
        acc = res.results[b * TP]["out"].astype(np.float32).copy()
        for lane in range(1, TP):
            acc += res.results[b * TP + lane]["out"]
        out[b] = acc + b_proj[None, :]
    return out
